# revision 59
# baseline (speedup 1.0000x reference)
"""Causal self-attention (RoPE) Trainium2 Bass kernel, 8-way sharded.

Sharding: core c handles batch c//4 and heads 4*(c%4) .. 4*(c%4)+4
(tensor-parallel over heads x data-parallel over batch). Each core
computes its QKV column shard, RoPE, causal attention for its 4 heads,
and a row-shard of the out-projection; the host sums the 4 partial
outputs per batch (the all-reduce realized at gather time).

fp8 DoubleRow strategy (v2): the PE's fp8 DoubleRow mode contracts two
128-row k-tiles per instruction at 0.5 cycles/row. Plain fp8 loses too
much precision (e4m3 ~2% rms -> ~3e-2 rel err per operand), so the two
pair slots carry an (hi, lo) residual split instead, giving ~11-bit
operands:
  - QKV/V matmuls: 3-term packing per dt k-tile pair:
      (w_hi,w_lo)^T(x_hi,x_hi)  one instr per dt   [w full, x hi]
      (w_hi[2p],w_hi[2p+1])^T(x_lo[2p],x_lo[2p+1]) one instr per pair
    -> 1.33x bf16 with ~bf16 accuracy.
  - PV: stationary (V_hi,V_lo), moving (pt,pt) stride-0 -> 2x.
  - softmax numerator pt is plain fp8 (the one affordable knob);
    exp(s - ln8) keeps pt <= ~30 (e4m3 max 240).
  - sums: ones-pair DR over co-located pt (i,i+1) pair tiles.
  - S and out-proj stay bf16.
Scales: w_qkv stored *16 (fp8 subnormal floor), folded back via
wo/16 on host; at = o/r lands at 16x and wo/16 cancels it.
"""

import sys

if "/opt/trn_rl_repo" not in sys.path:
    sys.path.insert(0, "/opt/trn_rl_repo")

import numpy as np
import ml_dtypes

import concourse.bass as bass
import concourse.mybir as mybir
import concourse.tile as tile
from concourse import bacc
from concourse.bass_utils import run_bass_kernel_spmd

FP32 = mybir.dt.float32
BF16 = mybir.dt.bfloat16
F8 = mybir.dt.float8e4
BF16_NP = ml_dtypes.bfloat16
F8_NP = ml_dtypes.float8_e4m3
DR = mybir.MatmulPerfMode.DoubleRow

B = 2
T = 2048
DIM = 2048
NUM_HEADS = 16
HEAD_DIM = 128
INNER = NUM_HEADS * HEAD_DIM
N_CORES = 8
NH = 4            # heads per core
P = 128           # partitions
TB = T // 512     # 4 t-blocks of 512 tokens
DT = DIM // P     # 16 d-tiles
KT = T // P       # 16 k-tiles of 128 tokens
WS = 16.0         # fp8 storage scale of w_qkv (and hence Q,K,V psums)
SCALE = 1.0 / float(np.sqrt(HEAD_DIM))
EXP_SCALE = SCALE / (WS * WS)
LN8 = float(np.log(8.0))

_CACHE = {}


def _build_nc(reps=1, opts=None):
    o = {
        "ps_qk": 6,
        "ps_s": 2,        # [128, 1024] pair tiles = 2 banks each
        "ps_r": 1,
        "ps_o": 1,
        "ps_y": 2,
        "lookahead": 6,   # pair-stages of S/exp ahead of PV/sums consumers
        "spread_outproj": True,
        "op_dr": True,    # out-proj via at/wo residual fp8 DoubleRow
        "mask_dve": False,
        "atlo_dve": False,
        "exp_pair": True,  # exp over [128,1024] s_ps pairs vs per-tile
        "yo_alt": False,   # alternate yo copies between ACT and DVE
    }
    if opts:
        o.update(opts)
    nc = bacc.Bacc(None, target_bir_lowering=False)

    xt_d = nc.declare_dram_parameter("xt", [TB, P, DT * 1024], F8, isOutput=False)
    wqk_d = nc.declare_dram_parameter("wqk", [P, 8 * DT * 256], F8, isOutput=False)
    wv_d = nc.declare_dram_parameter("wv", [P, DT * 1024], F8, isOutput=False)
    if o["op_dr"]:
        wo_d = nc.declare_dram_parameter("wo", [P, NH * 2 * DIM], F8, isOutput=False)
    else:
        wo_d = nc.declare_dram_parameter("wo", [P, NH * DIM], BF16, isOutput=False)
    cos_d = nc.declare_dram_parameter("cosT", [P, T], BF16, isOutput=False)
    sin_d = nc.declare_dram_parameter("sinT", [P, T], BF16, isOutput=False)
    mask_d = nc.declare_dram_parameter("mask", [P, P], F8, isOutput=False)
    out_d = nc.declare_dram_parameter("out", [T, DIM], FP32, isOutput=True)

    EXP = mybir.ActivationFunctionType.Exp

    def pair2(ap):
        # [P, 2n] -> [P, 2, n]
        return ap.rearrange("p (two m) -> p two m", two=2)

    def dup2(ap):
        # [P, n] -> [P, 2, n] stride-0 duplicate
        return ap.unsqueeze(1).broadcast_to([ap.shape[0], 2, ap.shape[-1]])

    with tile.TileContext(nc) as tc:
        with (
            tc.tile_pool(name="const", bufs=1) as cpool,
            tc.tile_pool(name="qkstore", bufs=1) as qkpool,
            tc.tile_pool(name="vstore", bufs=1) as vpool,
            tc.tile_pool(name="xt", bufs=10) as xtpool,
            tc.tile_pool(name="tmp", bufs=3) as tmp,
            tc.tile_pool(name="pt", bufs=8) as ptpool,
            tc.tile_pool(name="pts", bufs=3) as ptspool,
            tc.tile_pool(name="attnT", bufs=8) as atpool,
            tc.tile_pool(name="outb", bufs=4) as outpool,
        ):
            # --- constants ---
            wqk = cpool.tile([P, 8 * DT * 256], F8)
            wv = cpool.tile([P, DT * 1024], F8)
            if o["op_dr"]:
                wo = cpool.tile([P, NH * 2 * DIM], F8)
                # [P, h, hi/lo, DIM]
                wo_v = wo[:].rearrange("p (h two d) -> p h two d", h=NH, two=2)
            else:
                wo = cpool.tile([P, NH * DIM], BF16)
            cosT = cpool.tile([P, T], BF16)
            sinT = cpool.tile([P, T], BF16)
            tri = cpool.tile([P, P], F8)
            ones = cpool.tile([P, 256], F8)
            biasap = cpool.tile([P, 1], FP32)

            # DMA transfers serialize on the shared DMA device; interleave
            # wqk ct-chunks with xt(tb=0) so compute starts early
            CSZ = DT * 256
            xt0 = []
            for dp in range(DT // 2):
                nc.sync.dma_start(
                    wqk[:, dp * CSZ : (dp + 1) * CSZ],
                    wqk_d[:, dp * CSZ : (dp + 1) * CSZ],
                )
                xt_tile = xtpool.tile([P, 2048], F8, tag="xt")
                nc.sync.dma_start(xt_tile[:], xt_d[0, :, dp * 2048 : (dp + 1) * 2048])
                xt0.append(xt_tile)
            nc.sync.dma_start(cosT[:], cos_d[:])
            nc.sync.dma_start(sinT[:], sin_d[:])
            nc.sync.dma_start(wv[:], wv_d[:])
            nc.sync.dma_start(tri[:], mask_d[:])
            nc.sync.dma_start(wo[:], wo_d[:])
            nc.gpsimd.memset(ones[:], 1.0)
            nc.gpsimd.memset(biasap[:], -LN8)

            # persistent stores: Q,K post-rope [hd, T] bf16 per (q/k, head);
            # V (hi|lo) fp8 per k-tile: [128, kt*(4h*256)]
            qkstore = qkpool.tile([P, 8 * T], BF16)   # ct = (q h0..h3, k h0..h3)
            vstore = vpool.tile([P, KT * 1024], F8)

            # ---------------- QKV + RoPE phase ----------------
            for _rep in range(reps):
              with (
                tc.tile_pool(name="ps_qk", bufs=o["ps_qk"], space="PSUM") as ps_qk,
                tc.tile_pool(name="ps_v", bufs=2, space="PSUM") as ps_v,
              ):
                for tb in range(TB):
                    if tb == 0 and _rep == 0:
                        xt_t = xt0
                    else:
                        xt_t = []
                        for dp in range(DT // 2):
                            xt_tile = xtpool.tile([P, 2048], F8, tag="xt")
                            nc.sync.dma_start(
                                xt_tile[:], xt_d[tb, :, dp * 2048 : (dp + 1) * 2048]
                            )
                            xt_t.append(xt_tile)

                    def x_hi(dt):
                        t = xt_t[dt // 2]
                        return t[:, (dt % 2) * 1024 : (dt % 2) * 1024 + 512]

                    def x_lo_pair(dp):
                        # (x_lo[2dp], x_lo[2dp+1]) [P, 2, 512]
                        return pair2(xt_t[dp][:])[:, :, 512:1024]

                    def x_hi_pair_slice(dp, s):
                        # (x_hi[2dp], x_hi[2dp+1]) [P, 2, 128] token slice s
                        return pair2(xt_t[dp][:])[:, :, s * P : s * P + P]

                    def x_hilo_slice(dt, s):
                        # (x_hi[dt], x_lo[dt]) [P, 2, 128] token slice s
                        t = xt_t[dt // 2]
                        off = (dt % 2) * 1024
                        return pair2(t[:, off : off + 1024])[:, :, s * P : s * P + P]

                    def rope_and_store(ps, ct):
                        # qb = psum in bf16; qbr = partition-rotated by 64
                        # (single-input ACT copies may shift partitions; DVE
                        # tensor_tensor may not)
                        tbs = slice(tb * 512, (tb + 1) * 512)
                        qb = tmp.tile([P, 512], BF16, tag="qb")
                        nc.scalar.copy(qb[:], ps[:])
                        qbr = tmp.tile([P, 512], BF16, tag="qbr")
                        nc.sync.dma_start(qbr[0:64, :], qb[64:128, :])
                        nc.sync.dma_start(qbr[64:128, :], qb[0:64, :])
                        t1 = tmp.tile([P, 512], BF16, tag="t1")
                        nc.vector.tensor_mul(t1[:], qb[:], cosT[:, tbs])
                        t2 = tmp.tile([P, 512], BF16, tag="t2")
                        nc.vector.tensor_mul(t2[:], qbr[:], sinT[:, tbs])
                        nc.vector.tensor_add(
                            qkstore[:, ct * T + tb * 512 : ct * T + (tb + 1) * 512],
                            t1[:],
                            t2[:],
                        )

                    # Q,K c-tiles (3-term residual DR), dt-major in groups of 4.
                    # First block runs ct-major so compute starts as soon as
                    # the first wqk chunk lands. V tiles interleave between
                    # qk groups to keep PE busy while rope drains the psums.
                    ct_major = tb == 0 and _rep == 0

                    def emit_v(s):
                        psv = ps_v.tile([P, 512], FP32)
                        for dt in range(DT):
                            nc.tensor.matmul(
                                psv[:],
                                x_hilo_slice(dt, s),
                                dup2(wv[:, dt * 1024 : dt * 1024 + 512]),
                                start=(dt == 0), stop=False,
                                perf_mode=DR,
                            )
                        for dp in range(DT // 2):
                            nc.tensor.matmul(
                                psv[:],
                                x_hi_pair_slice(dp, s),
                                pair2(wv[:, 2 * dp * 1024 : (2 * dp + 2) * 1024])[
                                    :, :, 512:1024
                                ],
                                start=False, stop=(dp == DT // 2 - 1),
                                perf_mode=DR,
                            )
                        kt_idx = tb * 4 + s
                        # vstore layout per kt: 4h x (hi 128 | lo 128)
                        vhi = vstore[:, kt_idx * 1024 : (kt_idx + 1) * 1024]
                        vhi4 = vhi.rearrange("p (h two m) -> p h two m", h=4, two=2)
                        nc.scalar.copy(
                            vhi4[:, :, 0, :],
                            psv[:].rearrange("p (h m) -> p h m", h=4),
                        )
                        nc.vector.tensor_sub(
                            vhi4[:, :, 1, :],
                            psv[:].rearrange("p (h m) -> p h m", h=4),
                            vhi4[:, :, 0, :],
                        )

                    for grp in range(2):
                        pss = [
                            ps_qk.tile([P, 512], FP32, name="psqk", tag="psqk")
                            for _ in range(4)
                        ]
                        ci_dt = (
                            [(ci, dt) for ci in range(4) for dt in range(DT)]
                            if ct_major and grp == 0
                            else [(ci, dt) for dt in range(DT) for ci in range(4)]
                        )
                        for ci, dt in ci_dt:
                            ct = grp * 4 + ci
                            woff = (ct * DT + dt) * 256
                            nc.tensor.matmul(
                                pss[ci][:],
                                pair2(wqk[:, woff : woff + 256]),
                                dup2(x_hi(dt)),
                                start=(dt == 0), stop=False,
                                perf_mode=DR,
                            )
                        for dp in range(DT // 2):
                            for ci in range(4):
                                ct = grp * 4 + ci
                                woff = (ct * DT + 2 * dp) * 256
                                nc.tensor.matmul(
                                    pss[ci][:],
                                    pair2(wqk[:, woff : woff + 512])[:, :, 0:128],
                                    x_lo_pair(dp),
                                    start=False, stop=(dp == DT // 2 - 1),
                                    perf_mode=DR,
                                )
                        for ci in range(4):
                            rope_and_store(pss[ci], grp * 4 + ci)
                        # V tiles fill PE while rope drains the qk psums
                        emit_v(2 * grp)
                        emit_v(2 * grp + 1)

              # ---------------- attention + out-proj phase ----------------
              with (
                tc.tile_pool(name="ps_s", bufs=o["ps_s"], space="PSUM") as ps_s,
                tc.tile_pool(name="ps_r", bufs=o["ps_r"], space="PSUM") as ps_r,
                tc.tile_pool(name="ps_o", bufs=o["ps_o"], space="PSUM") as ps_o,
                tc.tile_pool(name="ps_y", bufs=o["ps_y"], space="PSUM") as ps_y,
              ):
                LA = o["lookahead"]
                pts = {}       # (j,h,ip) -> pt pair tile [P, 1024] fp8
                ros = {}       # (j,h) -> (r_ps, o_ps)
                at_tiles = {}  # (j,h) -> at

                # j descending: the big j=3 out-proj drains early, leaving
                # only j=0's small tail at the end
                stages = [
                    (j, h, ip)
                    for j in reversed(range(TB))
                    for h in range(NH)
                    for ip in range(2 * j + 2)
                ]

                def emit_s(key):
                    j, h, ip = key
                    qoff = h * T
                    koff = (NH + h) * T
                    pt = ptpool.tile([P, 1024], F8, name="pt", tag="pt")
                    diag = ip >= 2 * j
                    if o["exp_pair"]:
                        s_halves = None
                        s_ps = ps_s.tile([P, 1024], FP32, name="s_ps", tag="s_ps")
                    else:
                        s_halves = [
                            ps_s.tile([P, 512], FP32, name="s_ps", tag="s_ps")
                            for _ in range(2)
                        ]

                    def s_slice(e, lo, hi):
                        if s_halves is not None:
                            return s_halves[e][:, lo:hi]
                        return s_ps[:, e * 512 + lo : e * 512 + hi]

                    for e in range(2):
                        i = 2 * ip + e
                        off = i - 4 * j if diag else 0
                        # diagonal tiles: only q >= i*128 is live; S, exp,
                        # and the pt tile are restricted to that column range
                        nc.tensor.matmul(
                            s_slice(e, off * P, 512),
                            qkstore[:, koff + i * P : koff + (i + 1) * P],
                            qkstore[:, qoff + j * 512 + off * P : qoff + (j + 1) * 512],
                            start=True,
                            stop=True,
                        )
                    if diag:
                        for e in range(2):
                            off = 2 * ip + e - 4 * j
                            if off > 0:
                                nc.gpsimd.memset(
                                    pt[:, e * 512 : e * 512 + off * P], 0.0
                                )
                            # triangle chunk via scratch + mask
                            sc = ptspool.tile([P, P], F8, name="ptsc", tag="ptsc")
                            nc.scalar.activation(
                                sc[:],
                                s_slice(e, off * P, (off + 1) * P),
                                EXP, bias=biasap[:], scale=EXP_SCALE,
                            )
                            mask_eng = nc.vector if o["mask_dve"] else nc.gpsimd
                            mask_eng.tensor_mul(
                                pt[:, e * 512 + off * P : e * 512 + (off + 1) * P],
                                sc[:],
                                tri[:],
                            )
                            if off < 3:
                                nc.scalar.activation(
                                    pt[:, e * 512 + (off + 1) * P : (e + 1) * 512],
                                    s_slice(e, (off + 1) * P, 512),
                                    EXP, bias=biasap[:], scale=EXP_SCALE,
                                )
                    elif o["exp_pair"]:
                        nc.scalar.activation(
                            pt[:], s_ps[:], EXP, bias=biasap[:], scale=EXP_SCALE
                        )
                    else:
                        for e in range(2):
                            nc.scalar.activation(
                                pt[:, e * 512 : (e + 1) * 512],
                                s_halves[e][:],
                                EXP, bias=biasap[:], scale=EXP_SCALE,
                            )
                    pts[key] = pt

                def emit_consume(key):
                    j, h, ip = key
                    n_ip = 2 * j + 2
                    pt = pts.pop(key)
                    if ip == 0:
                        o_ps = ps_o.tile([P, 512], FP32, name="o_ps", tag="o_ps")
                        r_ps = ps_r.tile([P, 512], FP32, name="r_ps", tag="r_ps")
                        ros[(j, h)] = (r_ps, o_ps)
                    r_ps, o_ps = ros[(j, h)]
                    diag = ip >= 2 * j
                    for e in range(2):
                        i = 2 * ip + e
                        stop = ip == n_ip - 1 and e == 1
                        # start/stop must cover the full psum region; the
                        # zeroed leading pt columns make full width safe there
                        off = i - 4 * j if diag and not stop else 0
                        voff = i * 1024 + h * 256
                        nc.tensor.matmul(
                            o_ps[:, off * P : 512],
                            pair2(vstore[:, voff : voff + 256]),
                            dup2(pt[:, e * 512 + off * P : (e + 1) * 512]),
                            start=(ip == 0 and e == 0),
                            stop=stop,
                            perf_mode=DR,
                        )
                    # sums over the pair; use the wider (even) slot's range —
                    # the odd slot's leading columns there are zeroed
                    rstop = ip == n_ip - 1
                    soff = (2 * ip - 4 * j) * P if diag and not rstop else 0
                    nc.tensor.matmul(
                        r_ps[:, soff:512],
                        pair2(ones[:]),
                        pair2(pt[:])[:, :, soff:512],
                        start=(ip == 0), stop=rstop,
                        perf_mode=DR,
                    )
                    if ip == n_ip - 1:
                        r_ps, o_ps = ros.pop((j, h))
                        rc = tmp.tile([P, 512], FP32, tag="rc")
                        nc.vector.reciprocal(rc[:], r_ps[:])
                        if o["op_dr"]:
                            # head-pair tile [P, (hi_e|lo_e|hi_o|lo_o)*512] fp8
                            if h % 2 == 0:
                                at = atpool.tile([P, 2048], F8, name="at", tag="at")
                                at_tiles[(j, h // 2)] = at
                            else:
                                at = at_tiles[(j, h // 2)]
                            hoff = (h % 2) * 1024
                            at_f = tmp.tile([P, 512], BF16, tag="at_f")
                            nc.vector.tensor_mul(at_f[:], o_ps[:], rc[:])
                            nc.vector.tensor_copy(
                                at[:, hoff : hoff + 512], at_f[:]
                            )
                            sub_eng = nc.vector if o["atlo_dve"] else nc.gpsimd
                            sub_eng.tensor_sub(
                                at[:, hoff + 512 : hoff + 1024],
                                at_f[:],
                                at[:, hoff : hoff + 512],
                            )
                        else:
                            at = atpool.tile([P, 512], BF16, name="at", tag="at")
                            nc.vector.tensor_mul(at[:], o_ps[:], rc[:])
                            at_tiles[(j, h)] = at
                        if h == NH - 1:
                            emit_outproj(j)

                y_pend = []

                def emit_y_group(j, at_j, s, e):
                    y_ps = ps_y.tile([P, 512], FP32, name="y_ps", tag="y_ps")
                    if o["op_dr"]:
                        # main: at(hi,lo) x dup(wo_hi) per head;
                        # cross: (at_hi_e, at_hi_o) x (wo_lo_e, wo_lo_o) per pair
                        for hp in range(2):
                            for par in range(2):
                                h = 2 * hp + par
                                nc.tensor.matmul(
                                    y_ps[:],
                                    pair2(
                                        at_j[hp][:, par * 1024 : par * 1024 + 1024]
                                    )[:, :, s * P : (s + 1) * P],
                                    dup2(wo_v[:, h, 0, e * 512 : (e + 1) * 512]),
                                    start=(h == 0), stop=False,
                                    perf_mode=DR,
                                )
                        for hp in range(2):
                            athi = at_j[hp][:].rearrange(
                                "p (two m) -> p two m", two=2
                            )[:, :, s * P : (s + 1) * P]
                            nc.tensor.matmul(
                                y_ps[:],
                                athi,
                                wo_v[:, 2 * hp : 2 * hp + 2, 1,
                                     e * 512 : (e + 1) * 512],
                                start=False, stop=(hp == 1),
                                perf_mode=DR,
                            )
                    else:
                        for h in range(NH):
                            nc.tensor.matmul(
                                y_ps[:],
                                at_j[h][:, s * P : (s + 1) * P],
                                wo[:, h * DIM + e * 512 : h * DIM + (e + 1) * 512],
                                start=(h == 0),
                                stop=(h == NH - 1),
                            )
                    yo = outpool.tile([P, 512], FP32, tag="yo")
                    if o["yo_alt"] and (s + e) % 2 == 0:
                        nc.scalar.copy(yo[:], y_ps[:])
                    else:
                        nc.vector.tensor_copy(yo[:], y_ps[:])
                    t0 = j * 512 + s * P
                    nc.sync.dma_start(
                        out_d[t0 : t0 + P, e * 512 : (e + 1) * 512], yo[:]
                    )

                def emit_outproj(j):
                    nh_keys = range(2) if o["op_dr"] else range(NH)
                    at_j = [at_tiles.pop((j, h)) for h in nh_keys]
                    groups = [(j, at_j, s, e) for s in range(4) for e in range(4)]
                    if o["spread_outproj"]:
                        y_pend.extend(groups)
                    else:
                        for g in groups:
                            emit_y_group(*g)

                for k in range(len(stages) + LA):
                    if k < len(stages):
                        emit_s(stages[k])
                    if k - LA >= 0:
                        emit_consume(stages[k - LA])
                    if y_pend:
                        emit_y_group(*y_pend.pop(0))
                while y_pend:
                    emit_y_group(*y_pend.pop(0))

    nc.compile()
    return nc


def _rope_tables():
    inv_freq = 1.0 / (
        10000.0 ** (np.arange(0, HEAD_DIM, 2, dtype=np.float32) / HEAD_DIM)
    )
    t = np.arange(T, dtype=np.float32)
    freqs = np.einsum("i,j->ij", t, inv_freq)          # [T, 64]
    emb = np.concatenate([freqs, freqs], axis=-1)      # [T, 128]
    cosT = np.cos(emb).T.astype(BF16_NP)               # [128, T]
    sinT = np.sin(emb).T                               # [128, T]
    sinS = np.concatenate([-sinT[:64], sinT[64:]], axis=0).astype(BF16_NP)
    return np.ascontiguousarray(cosT), np.ascontiguousarray(sinS)


def _split8(a):
    """fp32 array -> (hi, lo) fp8 e4m3 with lo the residual."""
    hi = a.astype(F8_NP)
    lo = (a - hi.astype(np.float32)).astype(F8_NP)
    return hi, lo


def prepare_inputs(x, w_qkv, w_out):
    x = np.asarray(x, dtype=np.float32)
    w_qkv = np.asarray(w_qkv, dtype=np.float32)
    w_out = np.asarray(w_out, dtype=np.float32)

    cosT, sinS = _rope_tables()

    # causal triangle for one 128x128 diagonal chunk (same for every k-tile)
    r_idx = np.arange(P)[:, None]
    c_idx = np.arange(P)[None, :]
    mask = (r_idx <= c_idx).astype(F8_NP)

    # per-batch x^T tiles: [TB, 128, DT*1024] with per-dt (hi|lo)
    xts = []
    for b in range(B):
        xT = np.ascontiguousarray(x[b].T)                          # [D, T] f32
        xth = xT.reshape(DT, P, TB, 512).transpose(2, 1, 0, 3)     # [TB,P,DT,512]
        hi, lo = _split8(xth)
        packed = np.stack([hi, lo], axis=3)                        # [TB,P,DT,2,512]
        xts.append(np.ascontiguousarray(packed.reshape(TB, P, DT * 1024)))

    # per head-group weight shards
    wqks, wvs, wos = [], [], []
    for g in range(4):
        h0 = NH * g
        cols = [w_qkv[:, 128 * (h0 + h) : 128 * (h0 + h + 1)] for h in range(NH)]
        cols += [
            w_qkv[:, INNER + 128 * (h0 + h) : INNER + 128 * (h0 + h + 1)]
            for h in range(NH)
        ]
        W = np.concatenate(cols, axis=1) * WS                       # [D, 8*128]
        Wr = W.reshape(DT, P, 8, P).transpose(1, 2, 0, 3)           # [P, 8, DT, 128]
        hi, lo = _split8(Wr)
        packed = np.stack([hi, lo], axis=3)                         # [P,8,DT,2,128]
        wqks.append(np.ascontiguousarray(packed.reshape(P, 8 * DT * 256)))

        WV = w_qkv[:, 2 * INNER + 128 * h0 : 2 * INNER + 128 * (h0 + NH)] * WS
        WVr = WV.reshape(DT, P, 512).transpose(1, 0, 2)             # [P, DT, 512]
        hi, lo = _split8(WVr)
        packed = np.stack([hi, lo], axis=2)                         # [P, DT, 2, 512]
        wvs.append(np.ascontiguousarray(packed.reshape(P, DT * 1024)))

        WO = w_out[128 * h0 : 128 * (h0 + NH), :] * WS              # [512, D]
        WOr = WO.reshape(NH, P, DIM).transpose(1, 0, 2)             # [P, NH, DIM]
        hi, lo = _split8(WOr)
        packed = np.stack([hi, lo], axis=2)                         # [P, NH, 2, DIM]
        wos.append(np.ascontiguousarray(packed.reshape(P, NH * 2 * DIM)))

    in_maps = []
    for c in range(N_CORES):
        b, g = divmod(c, 4)
        in_maps.append(
            {
                "xt": xts[b],
                "wqk": wqks[g],
                "wv": wvs[g],
                "wo": wos[g],
                "cosT": cosT,
                "sinT": sinS,
                "mask": mask,
            }
        )
    return in_maps


def kernel(x, w_qkv, w_out):
    in_maps = prepare_inputs(x, w_qkv, w_out)

    if "nc" not in _CACHE:
        _CACHE["nc"] = _build_nc()
    nc = _CACHE["nc"]

    res = run_bass_kernel_spmd(nc, in_maps, core_ids=list(range(N_CORES)))

    out = np.zeros((B, T, DIM), dtype=np.float32)
    for c in range(N_CORES):
        b = c // 4
        out[b] += res.results[c]["out"]
    # at (16x) @ wo (16x) -> fold the storage scales back out
    out *= 1.0 / (WS * WS)
    return out
